# revision 35
# baseline (speedup 1.0000x reference)
"""Trainium2 Bass kernel for AttentionBasedExperts MoE routing.

Math: out[e, b] = gate(env_index[b])[e] where gate(t) is a pure function of
the task id t in [0, 50).  The full MLP + softmax + top-2 collapses to a
[50, 16] gate table computed once per core; the per-sample work is a gather.

Per NeuronCore (8-way batch-parallel, 16384 samples):
  - gate table via transposed-layout MLP (no intermediate transposes):
    h1T = W1.T @ embT, h2T = W2.T @ h1T, logits = h2T.T @ W3; biases are
    per-partition ACT bias operands; softmax/top-2 renorm on DVE.
  - one-hot: env values broadcast to all partitions by DMA (fp16), then a
    single DVE is_equal against an iota column (4x perf mode) gives exact
    0/1 fp16; two sample-halves packed in partitions 0-63 / 64-127.
  - gather: out = gate2.T @ onehot, 16 PE matmuls of N=512, four chunks
    col-tiled into each PSUM bank; psum->sbuf copies split DVE/ACT;
    contiguous 256KB output DMAs; host de-interleaves the layout.
"""

import os
from contextlib import ExitStack

import numpy as np

import concourse.bass as bass
from concourse.bass import _add_dep_helper
import concourse.tile as tile
import concourse.mybir as mybir
from concourse import bacc
from concourse.masks import make_identity
from concourse.bass_utils import run_bass_kernel_spmd

F32 = mybir.dt.float32
F16 = mybir.dt.float16

N_CORES = 8
B = 131072
BS = B // N_CORES            # 16384 per NeuronCore
BS2 = BS // 2                # 8192 columns (2 sample-halves per column)
NT = 50                      # tasks
ED = 128                     # emb dim
HD = 256                     # hidden
NE = 16                      # experts
CHUNK = 512                  # psum-bank chunk (columns)
NCH = BS2 // CHUNK           # 16 chunks
NGRP = NCH // 4              # 4 chunk-groups (col-tiled psum packing)

AF = mybir.ActivationFunctionType
ALU = mybir.AluOpType


def build_nc() -> bass.Bass:
    nc = bacc.Bacc("TRN2", target_bir_lowering=False, debug=False)

    envf = nc.dram_tensor("envf", [128, BS2], F16, kind="ExternalInput")
    emb = nc.dram_tensor("emb_table", [NT, ED], F32, kind="ExternalInput")
    # host-packed weight blobs (one DMA each; laid out in SBUF tile order):
    # wba = [W1 (256) | b1 as 2 cols]
    # wbb = [W2[0:128] | W2[128:256] | W3[0:128] | W3[128:256] | b2 2 cols |
    #        b3 broadcast 16 cols]
    wba = nc.dram_tensor("wba", [128, 258], F32, kind="ExternalInput")
    wbb = nc.dram_tensor("wbb", [128, 562], F32, kind="ExternalInput")
    # raw gather layout: [group, 32*c + 16*s + e, j]; host de-interleaves
    out = nc.dram_tensor("out", [NGRP, 128, CHUNK], F32, kind="ExternalOutput")


    with ExitStack() as ctx:
        tc = ctx.enter_context(tile.TileContext(nc))
        sb = ctx.enter_context(tc.tile_pool(name="sb", bufs=1))
        resb = ctx.enter_context(tc.tile_pool(name="resb", bufs=4))
        tps = ctx.enter_context(tc.tile_pool(name="tps", bufs=1, space="PSUM"))
        tps2 = ctx.enter_context(tc.tile_pool(name="tps2", bufs=2, space="PSUM"))
        psr = ctx.enter_context(tc.tile_pool(name="psr", bufs=2, space="PSUM"))

        # ---- input DMAs ----
        # Everything on the SP ring in dependency order (emb first, then the
        # two packed weight blobs, then env); the ACT sequencer issues no
        # input DMAs so the table's relus/copies run as soon as their data
        # dependencies clear.
        with nc.named_scope("load"):
            emb_sb = sb.tile([NT, ED], F32)
            nc.sync.dma_start(out=emb_sb, in_=emb.ap())
            wba_sb = sb.tile([128, 258], F32)
            nc.scalar.dma_start(out=wba_sb, in_=wba.ap())
            wbb_sb = sb.tile([128, 562], F32)
            nc.scalar.dma_start(out=wbb_sb, in_=wbb.ap())

            env_bc = sb.tile([128, BS2], F16)
            for q in range(4):
                nc.sync.dma_start(
                    out=env_bc[:, q * (BS2 // 4):(q + 1) * (BS2 // 4)],
                    in_=envf.ap()[:, q * (BS2 // 4):(q + 1) * (BS2 // 4)],
                )

            # on-chip constants (no DMA completion latency)
            id_sb = sb.tile([NT, NT], F32)
            make_identity(nc, id_sb)
            iota_i = sb.tile([128, 1], mybir.dt.int32)
            nc.gpsimd.iota(out=iota_i, pattern=[[0, 1]], base=0, channel_multiplier=1)
            iota_f = sb.tile([128, 1], F32)
            nc.vector.tensor_copy(out=iota_f, in_=iota_i)
            ge64 = sb.tile([128, 1], F32)
            nc.vector.tensor_scalar(out=ge64, in0=iota_f, scalar1=64.0,
                                    scalar2=None, op0=ALU.is_ge)
            iota_sb = sb.tile([128, 1], F32)
            # iota_sb[p] = p %% 64; pad rows give 50..63 which never match an
            # env value (< 50), so no explicit -1 padding is needed
            nc.vector.scalar_tensor_tensor(out=iota_sb, in0=ge64, scalar=-64.0,
                                           in1=iota_f, op0=ALU.mult, op1=ALU.add)



        # ---- one-hot: exact 0/1 fp16 via DVE is_equal (4x mode) ----
        onehot = sb.tile([128, BS2], F16)

        def is_eq_chunk(q):
            return nc.vector.tensor_scalar(
                out=onehot[:, q * (BS2 // 4):(q + 1) * (BS2 // 4)],
                in0=env_bc[:, q * (BS2 // 4):(q + 1) * (BS2 // 4)],
                scalar1=iota_sb,
                scalar2=None,
                op0=ALU.is_equal,
            )

        with nc.named_scope("onehot"):
            is_eq_chunk(0)
            is_eq_chunk(1)
            is_eq_chunk(2)

        # ---- gate table ----
        with nc.named_scope("table"):
            embT_ps = tps.tile([ED, NT], F32, tag="tp", padded_shape=[128, 512])
            nc.tensor.transpose(embT_ps, emb_sb, id_sb)
            embT = sb.tile([ED, NT], F32)
            nc.scalar.copy(out=embT, in_=embT_ps)

            # h1T halves [128, 50] = relu(W1[:, h].T @ embT + b1[h])
            h1T = sb.tile([128, 2, NT], F32)
            for h in range(2):
                hp = tps2.tile([128, NT], F32, tag="hh", padded_shape=[128, 512])
                nc.tensor.matmul(hp, wba_sb[:, 128 * h:128 * (h + 1)], embT,
                                 start=True, stop=True)
                nc.scalar.activation(out=h1T[:, h, :], in_=hp, func=AF.Relu,
                                     bias=wba_sb[:, 256 + h:257 + h], scale=1.0)

            # h2T halves = relu(sum_a W2[a, h].T @ h1T[a] + b2[h]);
            # padded to 64 cols (zeros) so the logits matmuls are uniform M=64
            h2T = sb.tile([128, 2, 64], F32)
            nc.vector.memset(h2T, 0.0)
            for h in range(2):
                hp2 = tps2.tile([128, NT], F32, tag="h2", padded_shape=[128, 512])
                nc.tensor.matmul(hp2, wbb_sb[:, 128 * h:128 * (h + 1)], h1T[:, 0, :],
                                 start=True, stop=False)
                nc.tensor.matmul(hp2, wbb_sb[:, 256 + 128 * h:256 + 128 * (h + 1)], h1T[:, 1, :],
                                 start=False, stop=True)
                nc.scalar.activation(out=h2T[:, h, 0:NT], in_=hp2, func=AF.Relu,
                                     bias=wbb_sb[:, 544 + h:545 + h], scale=1.0)

            # logits [128, 16] = h2 @ W3 + b3, computed at partition blocks
            # 0-49 AND 64-113 (tile_position col 64) so the whole softmax and
            # the final gate write happen in both gather blocks at once
            lg_ps = tps.tile([128, 64], F32, tag="lg", padded_shape=[128, 512])
            for blk in range(2):
                pos = None if blk == 0 else (0, 64)
                dst = lg_ps[64 * blk:64 * (blk + 1), 0:NE]
                nc.tensor.matmul(dst, h2T[:, 0, :], wbb_sb[:, 512:528],
                                 start=True, stop=False, tile_position=pos)
                nc.tensor.matmul(dst, h2T[:, 1, :], wbb_sb[:, 528:544],
                                 start=False, stop=True, tile_position=pos)
            lg2 = sb.tile([128, NE], F32)
            nc.vector.tensor_tensor(out=lg2, in0=lg_ps[:, 0:NE], in1=wbb_sb[:, 546:562], op=ALU.add)

            # softmax + hard top-2 renormalize:
            # e = exp(logits - max); m1/m2 top-2 of e;
            # gate = e * (e >= m2) / (m1 + m2)  (softmax Z cancels)
            negmax = sb.tile([128, 1], F32)
            nc.vector.tensor_reduce(
                out=negmax, in_=lg2, axis=mybir.AxisListType.X, op=ALU.max, negate=True
            )
            e_sb = sb.tile([128, NE], F32)
            nc.scalar.activation(out=e_sb, in_=lg2, func=AF.Exp, bias=negmax, scale=1.0)

            m1 = sb.tile([128, 1], F32)
            nc.vector.tensor_reduce(out=m1, in_=e_sb, axis=mybir.AxisListType.X, op=ALU.max)
            ge1 = sb.tile([128, NE], F32)
            nc.vector.tensor_scalar(out=ge1, in0=e_sb, scalar1=m1, scalar2=None, op0=ALU.is_ge)
            e2 = sb.tile([128, NE], F32)
            nc.vector.scalar_tensor_tensor(
                out=e2, in0=ge1, scalar=-2.0, in1=e_sb, op0=ALU.mult, op1=ALU.add
            )
            m2 = sb.tile([128, 1], F32)
            nc.vector.tensor_reduce(out=m2, in_=e2, axis=mybir.AxisListType.X, op=ALU.max)

            s12 = sb.tile([128, 1], F32)
            nc.vector.tensor_tensor(out=s12, in0=m1, in1=m2, op=ALU.add)
            r12 = sb.tile([128, 1], F32)
            nc.vector.reciprocal(out=r12, in_=s12)

            mr = sb.tile([128, NE], F32)
            nc.vector.tensor_scalar(
                out=mr, in0=e_sb, scalar1=m2, scalar2=r12, op0=ALU.is_ge, op1=ALU.mult
            )

            # gate2 [128, 32] fp16: rows 0-49 cols 0-15 = gate (half A);
            # rows 64-113 cols 16-31 = gate (half B) - written directly since
            # the softmax ran in both partition blocks
            gate2 = sb.tile([128, 32], F16)
            nc.vector.memset(gate2, 0.0)
            nc.vector.tensor_tensor(
                out=gate2[0:NT, 0:NE], in0=mr[0:NT, :], in1=e_sb[0:NT, :], op=ALU.mult
            )
            gm2 = nc.vector.tensor_tensor(
                out=gate2[64:64 + NT, NE:2 * NE], in0=mr[64:64 + NT, :],
                in1=e_sb[64:64 + NT, :], op=ALU.mult
            )

        with nc.named_scope("onehot2"):
            # order behind the gate writes so the DVE runs the softmax chain
            # (the gather gate) before this last bulk compare
            ie = is_eq_chunk(3)
            _add_dep_helper(ie.ins, gm2.ins, sync=False,
                            reason="softmax before bulk is_eq")

        # ---- gather: 16 matmuls, col-tiled psum packing ----
        with nc.named_scope("gather"):
            for g in range(NGRP):
                res_ps = psr.tile([128, CHUNK], F32, tag="res")
                for c in range(4):
                    ch = 4 * g + c
                    nc.tensor.matmul(
                        res_ps[32 * c:32 * c + 32, :], gate2,
                        onehot[:, ch * CHUNK:(ch + 1) * CHUNK],
                        start=True, stop=True, tile_position=(0, 32 * c),
                    )
                res_sb = resb.tile([128, CHUNK], F32, tag="res_sb")
                if g % 2 == 0:
                    nc.vector.tensor_copy(out=res_sb, in_=res_ps)
                else:
                    nc.scalar.copy(out=res_sb, in_=res_ps)
                nc.sync.dma_start(out=out.ap()[g], in_=res_sb)

    nc.compile()
    return nc


_NC_CACHE = {}


def _get_nc() -> bass.Bass:
    if "nc" not in _NC_CACHE:
        _NC_CACHE["nc"] = build_nc()
    return _NC_CACHE["nc"]


def _env_f16(env_shard: np.ndarray) -> np.ndarray:
    # [128, BS2] fp16, pre-replicated: rows 0-63 = samples [0, BS2) (half A),
    # rows 64-127 = samples [BS2, BS) (half B)
    halves = env_shard.astype(np.float16).reshape(2, 1, BS2)
    return np.broadcast_to(halves, (2, 64, BS2)).reshape(128, BS2)


def _deinterleave(raw: np.ndarray) -> np.ndarray:
    # raw [NGRP, 128, CHUNK]; partition p = 32*c + 16*s + e;
    # sample b = s*BS2 + g*4*CHUNK + c*CHUNK + j
    a = raw.reshape(NGRP, 4, 2, NE, CHUNK)           # [g, c, s, e, j]
    a = a.transpose(3, 2, 0, 1, 4)                   # [e, s, g, c, j]
    return np.ascontiguousarray(a.reshape(NE, BS))


def kernel(**inputs) -> np.ndarray:
    env_index = np.asarray(inputs["env_index"]).astype(np.int64)
    W1 = np.asarray(inputs["W1"]).astype(np.float32)
    b1 = np.asarray(inputs["b1"]).astype(np.float32)
    W2 = np.asarray(inputs["W2"]).astype(np.float32)
    b2 = np.asarray(inputs["b2"]).astype(np.float32)
    W3 = np.asarray(inputs["W3"]).astype(np.float32)
    b3 = np.asarray(inputs["b3"]).astype(np.float32)
    shared = {
        "emb_table": np.ascontiguousarray(
            np.asarray(inputs["emb_table"]).astype(np.float32)),
        "wba": np.ascontiguousarray(
            np.concatenate([W1, b1.reshape(2, ED).T], axis=1)),
        "wbb": np.ascontiguousarray(np.concatenate(
            [W2[0:128], W2[128:256], W3[0:128], W3[128:256],
             b2.reshape(2, ED).T, np.broadcast_to(b3[None, :], (128, NE))],
            axis=1)),
    }
    assert env_index.shape == (B,)

    nc = _get_nc()
    in_maps = []
    for c in range(N_CORES):
        m = dict(shared)
        m["envf"] = np.ascontiguousarray(_env_f16(env_index[c * BS:(c + 1) * BS]))
        in_maps.append(m)

    trace = bool(int(os.environ.get("KERNEL_TRACE", "0")))
    res = run_bass_kernel_spmd(
        nc, in_maps, core_ids=list(range(N_CORES)), trace=trace,
    )
    if trace:
        kernel.last_exec_time_ns = res.exec_time_ns
        kernel.last_results = res
    shards = [_deinterleave(r["out"]) for r in res.results]
    full = np.concatenate(shards, axis=1)
    return full[:, :, None].astype(np.float32)


# revision 36
# speedup vs baseline: 1.1128x; 1.1128x over previous
"""Trainium2 Bass kernel for AttentionBasedExperts MoE routing.

Math: out[e, b] = gate(env_index[b])[e] where gate(t) is a pure function of
the task id t in [0, 50).  The full MLP + softmax + top-2 collapses to a
[50, 16] gate table computed once per core; the per-sample work is a gather.

Per NeuronCore (8-way batch-parallel, 16384 samples):
  - gate table via transposed-layout MLP (no intermediate transposes):
    h1T = W1.T @ embT, h2T = W2.T @ h1T, logits = h2T.T @ W3; biases are
    per-partition ACT bias operands; softmax/top-2 renorm on DVE.
  - one-hot: env values broadcast to all partitions by DMA (fp16), then a
    single DVE is_equal against an iota column (4x perf mode) gives exact
    0/1 fp16; two sample-halves packed in partitions 0-63 / 64-127.
  - gather: out = gate2.T @ onehot, 16 PE matmuls of N=512, four chunks
    col-tiled into each PSUM bank; psum->sbuf copies split DVE/ACT;
    contiguous 256KB output DMAs; host de-interleaves the layout.
"""

import os
from contextlib import ExitStack

import numpy as np

import concourse.bass as bass
from concourse.bass import _add_dep_helper
import concourse.tile as tile
import concourse.mybir as mybir
from concourse import bacc
from concourse.masks import make_identity
from concourse.bass_utils import run_bass_kernel_spmd

F32 = mybir.dt.float32
F16 = mybir.dt.float16

N_CORES = 8
B = 131072
BS = B // N_CORES            # 16384 per NeuronCore
BS2 = BS // 2                # 8192 columns (2 sample-halves per column)
NT = 50                      # tasks
ED = 128                     # emb dim
HD = 256                     # hidden
NE = 16                      # experts
CHUNK = 512                  # psum-bank chunk (columns)
NCH = BS2 // CHUNK           # 16 chunks
NGRP = NCH // 4              # 4 chunk-groups (col-tiled psum packing)

AF = mybir.ActivationFunctionType
ALU = mybir.AluOpType


def build_nc() -> bass.Bass:
    nc = bacc.Bacc("TRN2", target_bir_lowering=False, debug=False)

    envf = nc.dram_tensor("envf", [128, BS2], F16, kind="ExternalInput")
    emb = nc.dram_tensor("emb_table", [NT, ED], F32, kind="ExternalInput")
    w1 = nc.dram_tensor("W1", [ED, HD], F32, kind="ExternalInput")
    b1 = nc.dram_tensor("b1", [HD], F32, kind="ExternalInput")
    w2 = nc.dram_tensor("W2", [HD, HD], F32, kind="ExternalInput")
    b2 = nc.dram_tensor("b2", [HD], F32, kind="ExternalInput")
    w3 = nc.dram_tensor("W3", [HD, NE], F32, kind="ExternalInput")
    b3r = nc.dram_tensor("b3r", [128, NE], F32, kind="ExternalInput")
    # raw gather layout: [group, 32*c + 16*s + e, j]; host de-interleaves
    out = nc.dram_tensor("out", [NGRP, 128, CHUNK], F32, kind="ExternalOutput")


    with ExitStack() as ctx:
        tc = ctx.enter_context(tile.TileContext(nc))
        sb = ctx.enter_context(tc.tile_pool(name="sb", bufs=1))
        resb = ctx.enter_context(tc.tile_pool(name="resb", bufs=4))
        tps = ctx.enter_context(tc.tile_pool(name="tps", bufs=1, space="PSUM"))
        tps2 = ctx.enter_context(tc.tile_pool(name="tps2", bufs=2, space="PSUM"))
        psr = ctx.enter_context(tc.tile_pool(name="psr", bufs=2, space="PSUM"))

        # ---- input DMAs ----
        # Everything on the SP ring in dependency order (emb first, then the
        # two packed weight blobs, then env); the ACT sequencer issues no
        # input DMAs so the table's relus/copies run as soon as their data
        # dependencies clear.
        with nc.named_scope("load"):
            b3_bc = sb.tile([128, NE], F32)
            nc.gpsimd.dma_start(out=b3_bc, in_=b3r.ap())

            emb_sb = sb.tile([NT, ED], F32)
            nc.sync.dma_start(out=emb_sb, in_=emb.ap())
            w1_sb = sb.tile([ED, HD], F32)
            nc.sync.dma_start(out=w1_sb, in_=w1.ap())
            b1_sb = sb.tile([ED, 2], F32)
            nc.sync.dma_start(
                out=b1_sb, in_=b1.ap().rearrange("(a k) -> k a", a=2))
            w2_sb = sb.tile([128, 2, HD], F32)
            nc.sync.dma_start(
                out=w2_sb, in_=w2.ap().rearrange("(a k) n -> k a n", a=2))
            b2_sb = sb.tile([ED, 2], F32)
            nc.sync.dma_start(
                out=b2_sb, in_=b2.ap().rearrange("(a k) -> k a", a=2))
            w3_sb = sb.tile([128, 2, NE], F32)
            nc.sync.dma_start(
                out=w3_sb, in_=w3.ap().rearrange("(a k) n -> k a n", a=2))

            env_bc = sb.tile([128, BS2], F16)
            for q in range(4):
                nc.sync.dma_start(
                    out=env_bc[:, q * (BS2 // 4):(q + 1) * (BS2 // 4)],
                    in_=envf.ap()[:, q * (BS2 // 4):(q + 1) * (BS2 // 4)],
                )

            # on-chip constants (no DMA completion latency)
            id_sb = sb.tile([NT, NT], F32)
            make_identity(nc, id_sb)
            iota_i = sb.tile([128, 1], mybir.dt.int32)
            nc.gpsimd.iota(out=iota_i, pattern=[[0, 1]], base=0, channel_multiplier=1)
            iota_f = sb.tile([128, 1], F32)
            nc.vector.tensor_copy(out=iota_f, in_=iota_i)
            ge64 = sb.tile([128, 1], F32)
            nc.vector.tensor_scalar(out=ge64, in0=iota_f, scalar1=64.0,
                                    scalar2=None, op0=ALU.is_ge)
            iota_sb = sb.tile([128, 1], F32)
            # iota_sb[p] = p %% 64; pad rows give 50..63 which never match an
            # env value (< 50), so no explicit -1 padding is needed
            nc.vector.scalar_tensor_tensor(out=iota_sb, in0=ge64, scalar=-64.0,
                                           in1=iota_f, op0=ALU.mult, op1=ALU.add)



        # ---- one-hot: exact 0/1 fp16 via DVE is_equal (4x mode) ----
        onehot = sb.tile([128, BS2], F16)

        def is_eq_chunk(q):
            return nc.vector.tensor_scalar(
                out=onehot[:, q * (BS2 // 4):(q + 1) * (BS2 // 4)],
                in0=env_bc[:, q * (BS2 // 4):(q + 1) * (BS2 // 4)],
                scalar1=iota_sb,
                scalar2=None,
                op0=ALU.is_equal,
            )

        with nc.named_scope("onehot"):
            is_eq_chunk(0)
            is_eq_chunk(1)

        # ---- gate table ----
        with nc.named_scope("table"):
            embT_ps = tps.tile([ED, NT], F32, tag="tp", padded_shape=[128, 512])
            nc.tensor.transpose(embT_ps, emb_sb, id_sb)
            embT = sb.tile([ED, NT], F32)
            nc.scalar.copy(out=embT, in_=embT_ps)

            # h1T halves [128, 50] = relu(W1[:, h].T @ embT + b1[h])
            h1T = sb.tile([128, 2, NT], F32)
            for h in range(2):
                hp = tps2.tile([128, NT], F32, tag="hh", padded_shape=[128, 512])
                nc.tensor.matmul(hp, w1_sb[:, 128 * h:128 * (h + 1)], embT,
                                 start=True, stop=True)
                nc.scalar.activation(out=h1T[:, h, :], in_=hp, func=AF.Relu,
                                     bias=b1_sb[:, h:h + 1], scale=1.0)

            # h2T halves = relu(sum_a W2[a, h].T @ h1T[a] + b2[h]);
            # padded to 64 cols (zeros) so the logits matmuls are uniform M=64
            h2T = sb.tile([128, 2, 64], F32)
            nc.vector.memset(h2T, 0.0)
            for h in range(2):
                hp2 = tps2.tile([128, NT], F32, tag="h2", padded_shape=[128, 512])
                nc.tensor.matmul(hp2, w2_sb[:, 0, 128 * h:128 * (h + 1)], h1T[:, 0, :],
                                 start=True, stop=False)
                nc.tensor.matmul(hp2, w2_sb[:, 1, 128 * h:128 * (h + 1)], h1T[:, 1, :],
                                 start=False, stop=True)
                nc.scalar.activation(out=h2T[:, h, 0:NT], in_=hp2, func=AF.Relu,
                                     bias=b2_sb[:, h:h + 1], scale=1.0)

            # logits [128, 16] = h2 @ W3 + b3, computed at partition blocks
            # 0-49 AND 64-113 (tile_position col 64) so the whole softmax and
            # the final gate write happen in both gather blocks at once
            lg_ps = tps.tile([128, 64], F32, tag="lg", padded_shape=[128, 512])
            for blk in range(2):
                pos = None if blk == 0 else (0, 64)
                dst = lg_ps[64 * blk:64 * (blk + 1), 0:NE]
                nc.tensor.matmul(dst, h2T[:, 0, :], w3_sb[:, 0, :],
                                 start=True, stop=False, tile_position=pos)
                nc.tensor.matmul(dst, h2T[:, 1, :], w3_sb[:, 1, :],
                                 start=False, stop=True, tile_position=pos)
            lg2 = sb.tile([128, NE], F32)
            nc.vector.tensor_tensor(out=lg2, in0=lg_ps[:, 0:NE], in1=b3_bc, op=ALU.add)

            # softmax + hard top-2 renormalize:
            # e = exp(logits - max); m1/m2 top-2 of e;
            # gate = e * (e >= m2) / (m1 + m2)  (softmax Z cancels)
            negmax = sb.tile([128, 1], F32)
            nc.vector.tensor_reduce(
                out=negmax, in_=lg2, axis=mybir.AxisListType.X, op=ALU.max, negate=True
            )
            e_sb = sb.tile([128, NE], F32)
            nc.scalar.activation(out=e_sb, in_=lg2, func=AF.Exp, bias=negmax, scale=1.0)

            m1 = sb.tile([128, 1], F32)
            nc.vector.tensor_reduce(out=m1, in_=e_sb, axis=mybir.AxisListType.X, op=ALU.max)
            ge1 = sb.tile([128, NE], F32)
            nc.vector.tensor_scalar(out=ge1, in0=e_sb, scalar1=m1, scalar2=None, op0=ALU.is_ge)
            e2 = sb.tile([128, NE], F32)
            nc.vector.scalar_tensor_tensor(
                out=e2, in0=ge1, scalar=-2.0, in1=e_sb, op0=ALU.mult, op1=ALU.add
            )
            m2 = sb.tile([128, 1], F32)
            nc.vector.tensor_reduce(out=m2, in_=e2, axis=mybir.AxisListType.X, op=ALU.max)

            s12 = sb.tile([128, 1], F32)
            nc.vector.tensor_tensor(out=s12, in0=m1, in1=m2, op=ALU.add)
            r12 = sb.tile([128, 1], F32)
            nc.vector.reciprocal(out=r12, in_=s12)

            mr = sb.tile([128, NE], F32)
            nc.vector.tensor_scalar(
                out=mr, in0=e_sb, scalar1=m2, scalar2=r12, op0=ALU.is_ge, op1=ALU.mult
            )

            # gate2 [128, 32] fp16: rows 0-49 cols 0-15 = gate (half A);
            # rows 64-113 cols 16-31 = gate (half B) - written directly since
            # the softmax ran in both partition blocks
            gate2 = sb.tile([128, 32], F16)
            nc.vector.memset(gate2, 0.0)
            nc.vector.tensor_tensor(
                out=gate2[0:NT, 0:NE], in0=mr[0:NT, :], in1=e_sb[0:NT, :], op=ALU.mult
            )
            gm2 = nc.vector.tensor_tensor(
                out=gate2[64:64 + NT, NE:2 * NE], in0=mr[64:64 + NT, :],
                in1=e_sb[64:64 + NT, :], op=ALU.mult
            )

        with nc.named_scope("onehot2"):
            # order behind the gate writes so the DVE runs the softmax chain
            # (the gather gate) before these bulk compares
            for q in (2, 3):
                ie = is_eq_chunk(q)
                _add_dep_helper(ie.ins, gm2.ins, sync=False,
                                reason="softmax before bulk is_eq")

        # ---- gather: 16 matmuls, col-tiled psum packing ----
        with nc.named_scope("gather"):
            for g in range(NGRP):
                res_ps = psr.tile([128, CHUNK], F32, tag="res")
                for c in range(4):
                    ch = 4 * g + c
                    nc.tensor.matmul(
                        res_ps[32 * c:32 * c + 32, :], gate2,
                        onehot[:, ch * CHUNK:(ch + 1) * CHUNK],
                        start=True, stop=True, tile_position=(0, 32 * c),
                    )
                res_sb = resb.tile([128, CHUNK], F32, tag="res_sb")
                if g % 2 == 0:
                    nc.vector.tensor_copy(out=res_sb, in_=res_ps)
                else:
                    nc.scalar.copy(out=res_sb, in_=res_ps)
                nc.sync.dma_start(out=out.ap()[g], in_=res_sb)

    nc.compile()
    return nc


_NC_CACHE = {}


def _get_nc() -> bass.Bass:
    if "nc" not in _NC_CACHE:
        _NC_CACHE["nc"] = build_nc()
    return _NC_CACHE["nc"]


def _env_f16(env_shard: np.ndarray) -> np.ndarray:
    # [128, BS2] fp16, pre-replicated: rows 0-63 = samples [0, BS2) (half A),
    # rows 64-127 = samples [BS2, BS) (half B)
    halves = env_shard.astype(np.float16).reshape(2, 1, BS2)
    return np.broadcast_to(halves, (2, 64, BS2)).reshape(128, BS2)


def _deinterleave(raw: np.ndarray) -> np.ndarray:
    # raw [NGRP, 128, CHUNK]; partition p = 32*c + 16*s + e;
    # sample b = s*BS2 + g*4*CHUNK + c*CHUNK + j
    a = raw.reshape(NGRP, 4, 2, NE, CHUNK)           # [g, c, s, e, j]
    a = a.transpose(3, 2, 0, 1, 4)                   # [e, s, g, c, j]
    return np.ascontiguousarray(a.reshape(NE, BS))


def kernel(**inputs) -> np.ndarray:
    env_index = np.asarray(inputs["env_index"]).astype(np.int64)
    shared = {
        name: np.ascontiguousarray(np.asarray(inputs[name]).astype(np.float32))
        for name in ("emb_table", "W1", "b1", "W2", "b2", "W3")
    }
    b3 = np.asarray(inputs["b3"]).astype(np.float32)
    shared["b3r"] = np.ascontiguousarray(np.broadcast_to(b3[None, :], (128, NE)))
    assert env_index.shape == (B,)

    nc = _get_nc()
    in_maps = []
    for c in range(N_CORES):
        m = dict(shared)
        m["envf"] = np.ascontiguousarray(_env_f16(env_index[c * BS:(c + 1) * BS]))
        in_maps.append(m)

    trace = bool(int(os.environ.get("KERNEL_TRACE", "0")))
    res = run_bass_kernel_spmd(
        nc, in_maps, core_ids=list(range(N_CORES)), trace=trace,
    )
    if trace:
        kernel.last_exec_time_ns = res.exec_time_ns
        kernel.last_results = res
    shards = [_deinterleave(r["out"]) for r in res.results]
    full = np.concatenate(shards, axis=1)
    return full[:, :, None].astype(np.float32)
